# revision 1
# baseline (speedup 1.0000x reference)
import sys

sys.path.insert(0, "/opt/trn_rl_repo")

import numpy as np
import ml_dtypes

import concourse.bass as bass
import concourse.bacc as bacc
import concourse.mybir as mybir
import concourse.tile as tile
from concourse.ap import AP
from concourse.masks import make_identity
from concourse.bass_utils import run_bass_kernel_spmd

HIDDEN = 1024
HEADS = 16
HD = 64
B = 2
S = 2048
NCORES = 8
HPC = 4
NT = S // 128
L = 2175           # band length
W = L + 1          # dram pitch
BF = mybir.dt.bfloat16
F32 = mybir.dt.float32

_cached = {}


def build_nc():
    nc = bacc.Bacc("TRN2", target_bir_lowering=False, debug=False, num_devices=NCORES)
    hT = nc.declare_dram_parameter("hT", [HIDDEN, S], BF, isOutput=False)
    wqT = nc.declare_dram_parameter("wqT", [HIDDEN, 2 * 128], BF, isOutput=False)
    wkT = nc.declare_dram_parameter("wkT", [HIDDEN, 2 * 128], BF, isOutput=False)
    wvT = nc.declare_dram_parameter("wvT", [HIDDEN, HPC * HD], BF, isOutput=False)
    rT = nc.declare_dram_parameter("rT", [128, 4095], BF, isOutput=False)
    rrT = nc.declare_dram_parameter("rrT", [128, 4095], BF, isOutput=False)
    out = nc.declare_dram_parameter("out", [S, HPC * HD], F32, isOutput=True)

    with tile.TileContext(nc) as tc, \
         tc.tile_pool(name="cst", bufs=1) as cst, \
         tc.tile_pool(name="sb", bufs=2) as sb, \
         tc.tile_pool(name="dr", bufs=2, space="DRAM") as dr, \
         tc.tile_pool(name="ps", bufs=2, space="PSUM") as ps:

        ident = cst.tile([128, 128], BF, tag="ident")
        make_identity(nc, ident[:, :])

        h_sb = []
        for k in range(8):
            t = cst.tile([128, S], BF, tag=f"h{k}", name=f"h{k}")
            nc.sync.dma_start(out=t[:, :], in_=hT[k * 128:(k + 1) * 128, :])
            h_sb.append(t)
        r_sb = cst.tile([128, 4095], BF, tag="r")
        nc.sync.dma_start(out=r_sb[:, :], in_=rT[:, :])
        rr_sb = cst.tile([128, 4095], BF, tag="rr")
        nc.sync.dma_start(out=rr_sb[:, :], in_=rrT[:, :])
        wq_sb = cst.tile([128, 8 * 256], BF, tag="wq")
        wk_sb = cst.tile([128, 8 * 256], BF, tag="wk")
        wv_sb = cst.tile([128, 8 * 256], BF, tag="wv")
        for k in range(8):
            nc.sync.dma_start(out=wq_sb[:, k * 256:(k + 1) * 256], in_=wqT[k * 128:(k + 1) * 128, :])
            nc.sync.dma_start(out=wk_sb[:, k * 256:(k + 1) * 256], in_=wkT[k * 128:(k + 1) * 128, :])
            nc.sync.dma_start(out=wv_sb[:, k * 256:(k + 1) * 256], in_=wvT[k * 128:(k + 1) * 128, :])

        # ---- QKV projections ----
        qt = [cst.tile([128, S], BF, tag=f"qt{hp}", name=f"qt{hp}") for hp in range(2)]
        kt = [cst.tile([128, S], BF, tag=f"kt{hp}", name=f"kt{hp}") for hp in range(2)]
        for hp in range(2):
            for src_w, dst in ((wq_sb, qt[hp]), (wk_sb, kt[hp])):
                for ic in range(4):
                    pp = ps.tile([128, 512], F32, tag="sc", bufs=1, name="pp")
                    for k in range(8):
                        nc.tensor.matmul(
                            out=pp[:, :],
                            lhsT=src_w[:, k * 256 + hp * 128: k * 256 + hp * 128 + 128],
                            rhs=h_sb[k][:, ic * 512:(ic + 1) * 512],
                            start=(k == 0), stop=(k == 7))
                    nc.vector.tensor_copy(out=dst[:, ic * 512:(ic + 1) * 512], in_=pp[:, :])

        vones = [[cst.tile([128, 65], BF, tag=f"v{h}_{jt}", name=f"v{h}_{jt}")
                  for jt in range(NT)] for h in range(HPC)]
        for h in range(HPC):
            for jt in range(NT):
                nc.vector.memset(vones[h][jt][:, 64:65], 1.0)
            for jt in range(NT):
                pv = ps.tile([128, 64], F32, tag="sc", bufs=1, name="pv")
                for k in range(8):
                    nc.tensor.matmul(
                        out=pv[:, :],
                        lhsT=h_sb[k][:, jt * 128:(jt + 1) * 128],
                        rhs=wv_sb[:, k * 256 + h * 64: k * 256 + h * 64 + 64],
                        start=(k == 0), stop=(k == 7))
                nc.vector.tensor_copy(out=vones[h][jt][:, 0:64], in_=pv[:, :])

        def band_to_dram(lhs_ap, r_tile, base, ddst, ei, dst_off=0):
            """band [128, L] = lhs.T @ r[base:base+L] -> bf16 -> pitched dram write."""
            bs = sb.tile([128, L], BF, tag="bandsb", name="bandsb")
            for third in range(3):
                c0 = third * 725
                bp = ps.tile([128, 725], F32, tag="band", name="bp")
                nc.tensor.matmul(out=bp[:, 0:512], lhsT=lhs_ap,
                                 rhs=r_tile[:, base + c0:base + c0 + 512],
                                 start=True, stop=False)
                nc.tensor.matmul(out=bp[:, 512:725], lhsT=lhs_ap,
                                 rhs=r_tile[:, base + c0 + 512:base + c0 + 725],
                                 start=True, stop=True)
                if (ei + third) % 2 == 0:
                    nc.scalar.copy(out=bs[:, c0:c0 + 725], in_=bp[:, :])
                else:
                    nc.vector.tensor_copy(out=bs[:, c0:c0 + 725], in_=bp[:, :])
            nc.sync.dma_start(out=AP(ddst.tensor, ddst.offset + dst_off, [[W, 128], [1, L]]),
                              in_=bs[:, :])

        for h in range(HPC):
            hp, half = h // 2, h % 2
            qth, kth = qt[hp], kt[hp]
            d0 = half * 64

            pva = [ps.tile([128, 455], F32, tag="pva", name="pva", bufs=1),
                   ps.tile([128, 455], F32, tag="pvb", name="pvb", bufs=1),
                   ps.tile([128, 130], F32, tag="pvc", name="pvc", bufs=1)]

            def pv_slot(it):
                return pva[it // 7][:, (it % 7) * 65:(it % 7) * 65 + 65]

            # phase 1: all A-bands (q side, reversed table) into ONE overlapped
            # pitched DRAM buffer: flat[r*(W-1) + m] = q_r * rr[1920 - r + m].
            # Band `it` written at base (W-1)*128*it with pitch W; overlapping
            # ranges between consecutive bands store identical values.
            ADU = (W - 1) * 128 * (NT - 1) + 127 * W + L
            adu = dr.tile([ADU], BF, tag="adu", name="adu")
            for it in range(NT):
                band_to_dram(qth[d0:d0 + 64, it * 128:(it + 1) * 128], rr_sb[d0:d0 + 64, :],
                             1920 - it * 128, adu, it, dst_off=(W - 1) * 128 * it)

            for jt in range(NT):
                bd = dr.tile([128, W], BF, tag="bd", name="bd")
                band_to_dram(kth[d0:d0 + 64, jt * 128:(jt + 1) * 128], r_sb[d0:d0 + 64, :],
                             1920 - jt * 128, bd, jt)

                # tt = T1T (one big xbar transpose) += T2T (accum pitched read)
                tt = sb.tile([128, S], BF, tag="tt", name="tt")
                nc.sync.dma_start(
                    out=tt[:, :],
                    in_=AP(adu.tensor, adu.offset + 127 + jt * 128,
                           [[W - 1, S], [1, 128]]),
                    transpose=True)
                nc.gpsimd.dma_start(
                    out=tt[:, :],
                    in_=AP(bd.tensor, bd.offset + 127, [[L, 128], [1, S]]),
                    accum_op=mybir.AluOpType.add)

                for ic in range(4):
                    sc = ps.tile([128, 512], F32, tag="sc", bufs=1, name="sc")
                    nc.tensor.matmul(out=sc[:, :],
                                     lhsT=kth[d0:d0 + 64, jt * 128:(jt + 1) * 128],
                                     rhs=qth[d0:d0 + 64, ic * 512:(ic + 1) * 512],
                                     start=True, stop=False)
                    nc.tensor.matmul(out=sc[:, :], lhsT=ident[:, :],
                                     rhs=tt[:, ic * 512:(ic + 1) * 512],
                                     start=False, stop=True)
                    ex = sb.tile([128, 512], BF, tag="ex", name="ex")
                    nc.scalar.activation(ex[:, :], sc[:, :], mybir.ActivationFunctionType.Exp,
                                         bias=0.0, scale=0.125)
                    for b4 in range(4):
                        it = ic * 4 + b4
                        # start=True clears has_written for the WHOLE bank, so only
                        # the first slot of each bank may set it (slots 0, 7, 14).
                        nc.tensor.matmul(out=pv_slot(it),
                                         lhsT=ex[:, b4 * 128:(b4 + 1) * 128],
                                         rhs=vones[h][jt][:, :],
                                         start=(jt == 0 and it in (0, 7, 14)),
                                         stop=(jt == 15))

            for it in range(NT):
                zr = sb.tile([128, 1], F32, tag="zr", name="zr")
                nc.vector.reciprocal(out=zr[:, :], in_=pv_slot(it)[:, 64:65])
                ctx = sb.tile([128, 64], F32, tag="ctx", name="ctx")
                nc.vector.tensor_scalar(out=ctx[:, :], in0=pv_slot(it)[:, 0:64],
                                        scalar1=zr[:, :], scalar2=None,
                                        op0=mybir.AluOpType.mult)
                nc.sync.dma_start(out=out[it * 128:(it + 1) * 128, h * 64:(h + 1) * 64],
                                  in_=ctx[:, :])
    nc.compile()
    return nc


def kernel(hidden_states, Wq, bq, Wk, bk, Wv, bv, dist_emb, _trace=False):
    hidden_states = np.asarray(hidden_states, np.float32)
    Wq, Wk, Wv = (np.asarray(w, np.float32) for w in (Wq, Wk, Wv))
    dist_emb = np.asarray(dist_emb, np.float32)

    def bf(x):
        return np.ascontiguousarray(x.astype(ml_dtypes.bfloat16))

    dist8 = dist_emb * 8.0
    rT = bf(np.vstack([dist8.T, dist8.T]))
    rrT = bf(np.vstack([dist8[::-1].T, dist8[::-1].T]))

    in_maps = []
    for c in range(NCORES):
        b = c // 4
        h0 = (c % 4) * HPC
        in_maps.append({
            "hT": bf(hidden_states[b].T),
            "wqT": bf(Wq[h0 * HD:(h0 + HPC) * HD, :].T),
            "wkT": bf(Wk[h0 * HD:(h0 + HPC) * HD, :].T),
            "wvT": bf(Wv[h0 * HD:(h0 + HPC) * HD, :].T),
            "rT": rT, "rrT": rrT,
        })

    if "nc" not in _cached:
        _cached["nc"] = build_nc()
    nc = _cached["nc"]
    import time as _time
    res = run_bass_kernel_spmd(nc, in_maps, list(range(NCORES)))
    if _trace:
        times = []
        for _ in range(2):
            t0 = _time.perf_counter()
            res = run_bass_kernel_spmd(nc, in_maps, list(range(NCORES)))
            np.asarray(res.results[0]["out"])
            times.append(_time.perf_counter() - t0)
        print("HW exec time:", int(min(times) * 1e9), "ns  (wall of exec+transfer; runs:",
              [f"{t*1e3:.1f}ms" for t in times], ")")
        _cached["exec_ns"] = int(min(times) * 1e9)

    outs = [np.asarray(res.results[c]["out"]) for c in range(NCORES)]
    full = np.zeros((B, S, HEADS, HD), np.float32)
    for c in range(NCORES):
        b = c // 4
        h0 = (c % 4) * HPC
        full[b, :, h0:h0 + HPC, :] = outs[c].reshape(S, HPC, HD)
    return full.reshape(B, S, HEADS * HD)



# revision 2
# speedup vs baseline: 2008.2097x; 2008.2097x over previous
import sys

sys.path.insert(0, "/opt/trn_rl_repo")

import numpy as np
import ml_dtypes

import concourse.bass as bass
import concourse.bacc as bacc
import concourse.mybir as mybir
import concourse.tile as tile
from concourse.ap import AP
from concourse.masks import make_identity

HIDDEN = 1024
HEADS = 16
HD = 64
B = 2
S = 2048
NCORES = 8
HPC = 4
NT = S // 128
L = 2175           # band length
W = L + 1          # dram pitch
BF = mybir.dt.bfloat16
F32 = mybir.dt.float32

_cached = {}


def build_nc(reps=1):
    nc = bacc.Bacc("TRN2", target_bir_lowering=False, debug=False, num_devices=NCORES)
    hT = nc.declare_dram_parameter("hT", [HIDDEN, S], BF, isOutput=False)
    wqT = nc.declare_dram_parameter("wqT", [HIDDEN, 2 * 128], BF, isOutput=False)
    wkT = nc.declare_dram_parameter("wkT", [HIDDEN, 2 * 128], BF, isOutput=False)
    wvT = nc.declare_dram_parameter("wvT", [HIDDEN, HPC * HD], BF, isOutput=False)
    rT = nc.declare_dram_parameter("rT", [128, 4095], BF, isOutput=False)
    rrT = nc.declare_dram_parameter("rrT", [128, 4095], BF, isOutput=False)
    out = nc.declare_dram_parameter("out", [S, HPC * HD], F32, isOutput=True)

    with tile.TileContext(nc) as tc, \
         tc.tile_pool(name="cst", bufs=1) as cst, \
         tc.tile_pool(name="sb", bufs=2) as sb, \
         tc.tile_pool(name="dr", bufs=2, space="DRAM") as dr, \
         tc.tile_pool(name="ps", bufs=2, space="PSUM") as ps:

        ident = cst.tile([128, 128], BF, tag="ident")
        make_identity(nc, ident[:, :])

        r_sb = cst.tile([128, 4095], BF, tag="r")
        nc.sync.dma_start(out=r_sb[:, :], in_=rT[:, :])
        rr_sb = cst.tile([128, 4095], BF, tag="rr")
        nc.sync.dma_start(out=rr_sb[:, :], in_=rrT[:, :])

        for _rep in range(reps):
            h_sb = []
            for k in range(8):
                t = cst.tile([128, S], BF, tag=f"h{k}", name=f"h{k}")
                nc.sync.dma_start(out=t[:, :], in_=hT[k * 128:(k + 1) * 128, :])
                h_sb.append(t)
            wq_sb = cst.tile([128, 8 * 256], BF, tag="wq")
            wk_sb = cst.tile([128, 8 * 256], BF, tag="wk")
            wv_sb = cst.tile([128, 8 * 256], BF, tag="wv")
            for k in range(8):
                nc.sync.dma_start(out=wq_sb[:, k * 256:(k + 1) * 256], in_=wqT[k * 128:(k + 1) * 128, :])
                nc.sync.dma_start(out=wk_sb[:, k * 256:(k + 1) * 256], in_=wkT[k * 128:(k + 1) * 128, :])
                nc.sync.dma_start(out=wv_sb[:, k * 256:(k + 1) * 256], in_=wvT[k * 128:(k + 1) * 128, :])

            # ---- QKV projections ----
            qt = [cst.tile([128, S], BF, tag=f"qt{hp}", name=f"qt{hp}") for hp in range(2)]
            kt = [cst.tile([128, S], BF, tag=f"kt{hp}", name=f"kt{hp}") for hp in range(2)]
            for hp in range(2):
                for src_w, dst in ((wq_sb, qt[hp]), (wk_sb, kt[hp])):
                    for ic in range(4):
                        pp = ps.tile([128, 512], F32, tag="sc", bufs=1, name="pp")
                        for k in range(8):
                            nc.tensor.matmul(
                                out=pp[:, :],
                                lhsT=src_w[:, k * 256 + hp * 128: k * 256 + hp * 128 + 128],
                                rhs=h_sb[k][:, ic * 512:(ic + 1) * 512],
                                start=(k == 0), stop=(k == 7))
                        nc.vector.tensor_copy(out=dst[:, ic * 512:(ic + 1) * 512], in_=pp[:, :])

            vones = [[cst.tile([128, 65], BF, tag=f"v{h}_{jt}", name=f"v{h}_{jt}")
                      for jt in range(NT)] for h in range(HPC)]
            for h in range(HPC):
                for jt in range(NT):
                    nc.vector.memset(vones[h][jt][:, 64:65], 1.0)
                for jt in range(NT):
                    pv = ps.tile([128, 64], F32, tag="sc", bufs=1, name="pv")
                    for k in range(8):
                        nc.tensor.matmul(
                            out=pv[:, :],
                            lhsT=h_sb[k][:, jt * 128:(jt + 1) * 128],
                            rhs=wv_sb[:, k * 256 + h * 64: k * 256 + h * 64 + 64],
                            start=(k == 0), stop=(k == 7))
                    nc.vector.tensor_copy(out=vones[h][jt][:, 0:64], in_=pv[:, :])

            def band_to_dram(lhs_ap, r_tile, base, ddst, ei, dst_off=0):
                """band [128, L] = lhs.T @ r[base:base+L] -> bf16 -> pitched dram write."""
                bs = sb.tile([128, L], BF, tag="bandsb", name="bandsb")
                for third in range(3):
                    c0 = third * 725
                    bp = ps.tile([128, 725], F32, tag="band", name="bp")
                    nc.tensor.matmul(out=bp[:, 0:512], lhsT=lhs_ap,
                                     rhs=r_tile[:, base + c0:base + c0 + 512],
                                     start=True, stop=False)
                    nc.tensor.matmul(out=bp[:, 512:725], lhsT=lhs_ap,
                                     rhs=r_tile[:, base + c0 + 512:base + c0 + 725],
                                     start=True, stop=True)
                    if (ei + third) % 2 == 0:
                        nc.scalar.copy(out=bs[:, c0:c0 + 725], in_=bp[:, :])
                    else:
                        nc.vector.tensor_copy(out=bs[:, c0:c0 + 725], in_=bp[:, :])
                nc.sync.dma_start(out=AP(ddst.tensor, ddst.offset + dst_off, [[W, 128], [1, L]]),
                                  in_=bs[:, :])

            for h in range(HPC):
                hp, half = h // 2, h % 2
                qth, kth = qt[hp], kt[hp]
                d0 = half * 64

                pva = [ps.tile([128, 455], F32, tag="pva", name="pva", bufs=1),
                       ps.tile([128, 455], F32, tag="pvb", name="pvb", bufs=1),
                       ps.tile([128, 130], F32, tag="pvc", name="pvc", bufs=1)]

                def pv_slot(it):
                    return pva[it // 7][:, (it % 7) * 65:(it % 7) * 65 + 65]

                # phase 1: all A-bands (q side, reversed table) into ONE overlapped
                # pitched DRAM buffer: flat[r*(W-1) + m] = q_r * rr[1920 - r + m].
                # Band `it` written at base (W-1)*128*it with pitch W; overlapping
                # ranges between consecutive bands store identical values.
                ADU = (W - 1) * 128 * (NT - 1) + 127 * W + L
                adu = dr.tile([ADU], BF, tag="adu", name="adu")
                for it in range(NT):
                    band_to_dram(qth[d0:d0 + 64, it * 128:(it + 1) * 128], rr_sb[d0:d0 + 64, :],
                                 1920 - it * 128, adu, it, dst_off=(W - 1) * 128 * it)

                for jt in range(NT):
                    bd = dr.tile([128, W], BF, tag="bd", name="bd")
                    band_to_dram(kth[d0:d0 + 64, jt * 128:(jt + 1) * 128], r_sb[d0:d0 + 64, :],
                                 1920 - jt * 128, bd, jt)

                    # tt = T1T (one big xbar transpose) += T2T (accum pitched read)
                    tt = sb.tile([128, S], BF, tag="tt", name="tt")
                    nc.sync.dma_start(
                        out=tt[:, :],
                        in_=AP(adu.tensor, adu.offset + 127 + jt * 128,
                               [[W - 1, S], [1, 128]]),
                        transpose=True)
                    nc.gpsimd.dma_start(
                        out=tt[:, :],
                        in_=AP(bd.tensor, bd.offset + 127, [[L, 128], [1, S]]),
                        accum_op=mybir.AluOpType.add)

                    for ic in range(4):
                        sc = ps.tile([128, 512], F32, tag="sc", bufs=1, name="sc")
                        nc.tensor.matmul(out=sc[:, :],
                                         lhsT=kth[d0:d0 + 64, jt * 128:(jt + 1) * 128],
                                         rhs=qth[d0:d0 + 64, ic * 512:(ic + 1) * 512],
                                         start=True, stop=False)
                        nc.tensor.matmul(out=sc[:, :], lhsT=ident[:, :],
                                         rhs=tt[:, ic * 512:(ic + 1) * 512],
                                         start=False, stop=True)
                        ex = sb.tile([128, 512], BF, tag="ex", name="ex")
                        nc.scalar.activation(ex[:, :], sc[:, :], mybir.ActivationFunctionType.Exp,
                                             bias=0.0, scale=0.125)
                        for b4 in range(4):
                            it = ic * 4 + b4
                            # start=True clears has_written for the WHOLE bank, so only
                            # the first slot of each bank may set it (slots 0, 7, 14).
                            nc.tensor.matmul(out=pv_slot(it),
                                             lhsT=ex[:, b4 * 128:(b4 + 1) * 128],
                                             rhs=vones[h][jt][:, :],
                                             start=(jt == 0 and it in (0, 7, 14)),
                                             stop=(jt == 15))

                for it in range(NT):
                    zr = sb.tile([128, 1], F32, tag="zr", name="zr")
                    nc.vector.reciprocal(out=zr[:, :], in_=pv_slot(it)[:, 64:65])
                    ctx = sb.tile([128, 64], F32, tag="ctx", name="ctx")
                    nc.vector.tensor_scalar(out=ctx[:, :], in0=pv_slot(it)[:, 0:64],
                                            scalar1=zr[:, :], scalar2=None,
                                            op0=mybir.AluOpType.mult)
                    nc.sync.dma_start(out=out[it * 128:(it + 1) * 128, h * 64:(h + 1) * 64],
                                      in_=ctx[:, :])
    nc.compile()
    return nc


def _make_runner(nc):
    """Build a cached jit-compiled 8-core dispatcher for a compiled Bass module.

    Mirrors bass_utils.run_bass_kernel_spmd's axon path (bass2jax shard_map over
    _bass_exec_p), hoisted so the jit executable is built once and reused.
    """
    import jax
    from jax.sharding import Mesh, PartitionSpec, NamedSharding
    from jax.experimental.shard_map import shard_map
    from concourse.bass2jax import _bass_exec_p, install_neuronx_cc_hook, partition_id_tensor

    install_neuronx_cc_hook()

    partition_name = nc.partition_id_tensor.name if nc.partition_id_tensor else None
    in_names, out_names, out_avals = [], [], []
    for alloc in nc.m.functions[0].allocations:
        if not isinstance(alloc, mybir.MemoryLocationSet):
            continue
        name = alloc.memorylocations[0].name
        if alloc.kind == "ExternalInput":
            if name != partition_name:
                in_names.append(name)
        elif alloc.kind == "ExternalOutput":
            out_names.append(name)
            out_avals.append(jax.core.ShapedArray(
                tuple(alloc.tensor_shape), mybir.dt.np(alloc.dtype)))
    n_params = len(in_names)
    n_outs = len(out_avals)
    in_names_all = in_names + out_names
    if partition_name is not None:
        in_names_all.append(partition_name)
    donate = tuple(range(n_params, n_params + n_outs))

    def _body(*args):
        operands = list(args)
        if partition_name is not None:
            operands.append(partition_id_tensor())
        outs = _bass_exec_p.bind(
            *operands,
            out_avals=tuple(out_avals), in_names=tuple(in_names_all),
            out_names=tuple(out_names), lowering_input_output_aliases=(),
            sim_require_finite=True, sim_require_nnan=True, nc=nc,
        )
        return tuple(outs)

    devices = jax.devices()[:NCORES]
    mesh = Mesh(np.asarray(devices), ("core",))
    fn = jax.jit(
        shard_map(_body, mesh=mesh,
                  in_specs=(PartitionSpec("core"),) * (n_params + n_outs),
                  out_specs=(PartitionSpec("core"),) * n_outs, check_rep=False),
        donate_argnums=donate, keep_unused=True)
    shard = NamedSharding(mesh, PartitionSpec("core"))
    return {
        "fn": fn, "in_names": in_names, "out_names": out_names,
        "out_avals": out_avals, "shard": shard, "jax": jax,
    }


def get_runner(reps=1):
    key = f"run{reps}"
    if key not in _cached:
        _cached[key] = _make_runner(build_nc(reps))
    return _cached[key]


def prepare_in_maps(hidden_states, Wq, bq, Wk, bk, Wv, bv, dist_emb):
    hidden_states = np.asarray(hidden_states, np.float32)
    Wq, Wk, Wv = (np.asarray(w, np.float32) for w in (Wq, Wk, Wv))
    dist_emb = np.asarray(dist_emb, np.float32)

    def bf(x):
        return np.ascontiguousarray(x.astype(ml_dtypes.bfloat16))

    dist8 = dist_emb * 8.0
    rT = bf(np.vstack([dist8.T, dist8.T]))
    rrT = bf(np.vstack([dist8[::-1].T, dist8[::-1].T]))

    hTb = [bf(hidden_states[b].T) for b in range(B)]
    in_maps = []
    for c in range(NCORES):
        b = c // 4
        h0 = (c % 4) * HPC
        in_maps.append({
            "hT": hTb[b],
            "wqT": bf(Wq[h0 * HD:(h0 + HPC) * HD, :].T),
            "wkT": bf(Wk[h0 * HD:(h0 + HPC) * HD, :].T),
            "wvT": bf(Wv[h0 * HD:(h0 + HPC) * HD, :].T),
            "rT": rT, "rrT": rrT,
        })
    return in_maps


def stage_inputs(runner, in_maps):
    """Concat per-core inputs and place on devices. Returns device arrays."""
    jax = runner["jax"]
    dev_in = [
        jax.device_put(
            np.concatenate([np.asarray(in_maps[c][nm]) for c in range(NCORES)], axis=0),
            runner["shard"])
        for nm in runner["in_names"]
    ]
    for x in dev_in:
        x.block_until_ready()
    return dev_in


def fresh_out(runner):
    jax = runner["jax"]
    av = runner["out_avals"][0]
    z = jax.device_put(np.zeros((NCORES * av.shape[0], *av.shape[1:]), av.dtype),
                       runner["shard"])
    z.block_until_ready()
    return z


def unshard(host_out):
    """[NCORES*S, HPC*HD] concat -> full [B, S, HIDDEN]."""
    arr = np.asarray(host_out).reshape(NCORES, S, HPC * HD)
    full = np.zeros((B, S, HEADS, HD), np.float32)
    for c in range(NCORES):
        b = c // 4
        h0 = (c % 4) * HPC
        full[b, :, h0:h0 + HPC, :] = arr[c].reshape(S, HPC, HD)
    return full.reshape(B, S, HEADS * HD)


def kernel(hidden_states, Wq, bq, Wk, bk, Wv, bv, dist_emb):
    in_maps = prepare_in_maps(hidden_states, Wq, bq, Wk, bk, Wv, bv, dist_emb)
    runner = get_runner(1)
    dev_in = stage_inputs(runner, in_maps)
    out = runner["fn"](*dev_in, fresh_out(runner))[0]
    return unshard(out)
